# Initial kernel scaffold
#
"""Bilateral filter (nn_BilateralFilter) on 8 Trainium2 NeuronCores.

Sharding: data-parallel over (batch, H-half): core i -> sample i//2,
row-half i%2 (128 output rows each). Each core receives a host-padded
input slab [C, 132, 260] (2-row/2-col zero halos) plus per-sample tap
weights derived from `params` on the host; it computes the 5x5 (or
masked 3x3) bilateral filter for its 128x256 output tile.

Math (exact rewrite of the reference):
  out[c,p] = sum_t sk[t]*e_t[p]*x[c,p+t] / sum_t (sk[t]+1e-8*mask[t])*e_t[p]
  e_t[p]   = exp(-((m*s)[p+t] - (m*s)[p])^2),  s = 1/(sqrt(2)*sigma2)
where m is the channel-mean image and sk is the mask-folded normalized
spatial kernel. The 1e-8*mask term reproduces the reference's
`w/(w.sum()+1e-8)` epsilon after multiplying through by the color-kernel
normalizer.
"""

import numpy as np

B, C, H, W = 4, 32, 256, 256
HALF = H // 2          # output rows per core
SLAB_H = HALF + 4      # input rows incl. 2-row halos
SLAB_W = W + 4         # input cols incl. 2-col halos
NCORES = 8
NT = 25                # 5x5 taps
CG = 4                 # channel groups
GC = C // CG           # channels per group

_CACHE = {}


def _host_tap_constants(params):
    """Per-sample sk_eff[25], sk2[25], s2c scalar (all float32 math)."""
    p = params.astype(np.float32)
    sig = (1.0 / (1.0 + np.exp(-p))).astype(np.float32)
    coords = (np.arange(5, dtype=np.float32) - 2.0)
    grid = coords[:, None] ** 2 + coords[None, :] ** 2
    center3 = ((np.abs(coords)[:, None] <= 1) & (np.abs(coords)[None, :] <= 1)).astype(np.float32)
    out = []
    for b in range(B):
        k_raw = np.float32(1.0) + np.float32(2.0) * sig[b, 0]
        is5 = bool(k_raw >= 2.0)
        sigma1 = np.float32(3.5) + np.float32(5.5) * sig[b, 1]
        sigma2 = np.float32(5.5) + np.float32(7.5) * sig[b, 2]
        mask = np.ones((5, 5), np.float32) if is5 else center3
        sk = np.exp(-grid / (2.0 * sigma1 ** 2)).astype(np.float32) * mask
        sk = (sk / sk.sum()).astype(np.float32)
        sk_eff = sk.reshape(NT)
        sk2 = (sk_eff + np.float32(1e-8) * mask.reshape(NT)).astype(np.float32)
        # m_s = (sum_c x) * s2c  ==  mean * (1/(sqrt(2)*sigma2))
        s2c = np.float32(1.0 / (np.sqrt(2.0, dtype=np.float64) * float(sigma2)) / C)
        out.append((sk_eff, sk2, s2c, is5))
    return out


def _build(active_taps):
    from contextlib import ExitStack
    import concourse.tile as tile
    from concourse import bacc, mybir

    f32 = mybir.dt.float32
    AF = mybir.ActivationFunctionType
    AL = mybir.AluOpType

    nc = bacc.Bacc("TRN2", target_bir_lowering=False, debug=False,
                   num_devices=NCORES)
    xs_d = nc.dram_tensor("xs", [C, SLAB_H, SLAB_W], f32, kind="ExternalInput").ap()
    cst_d = nc.dram_tensor("cst", [128, 51], f32, kind="ExternalInput").ap()
    out_d = nc.dram_tensor("out", [C, HALF, W], f32, kind="ExternalOutput").ap()

    with tile.TileContext(nc) as tc, ExitStack() as ctx:
        pool_c = ctx.enter_context(tc.tile_pool(name="cstp", bufs=1))
        pool_x = ctx.enter_context(tc.tile_pool(name="xp", bufs=2))
        pool_mean = ctx.enter_context(tc.tile_pool(name="meanp", bufs=1))
        pool_w = ctx.enter_context(tc.tile_pool(name="wp", bufs=3))
        pool_acc = ctx.enter_context(tc.tile_pool(name="accp", bufs=1))
        pool_tmp = ctx.enter_context(tc.tile_pool(name="tmpp", bufs=3))

        cst = pool_c.tile([128, 51], f32)
        nc.sync.dma_start(cst[:], cst_d)

        # ---- load x slabs for di=0 (also feeds the mean) + tail rows ----
        xg = {}
        for g in range(CG):
            t = pool_x.tile([128, GC, SLAB_W], f32, tag=f"xg{g}")
            nc.sync.dma_start(
                t[:], xs_d[g * GC:(g + 1) * GC, 0:128, :].transpose([1, 0, 2]))
            xg[(0, g)] = t
        xt = []
        for g in range(CG):
            t = pool_mean.tile([4, GC, SLAB_W], f32, tag=f"xt{g}")
            nc.sync.dma_start(
                t[:], xs_d[g * GC:(g + 1) * GC, 128:132, :].transpose([1, 0, 2]))
            xt.append(t)

        # ---- channel mean (rows 0..128 and tail rows 128..132) ----
        m_acc = pool_mean.tile([128, SLAB_W], f32)
        mt_acc = pool_mean.tile([4, SLAB_W], f32)
        for g in range(CG):
            if g == 0:
                nc.vector.tensor_reduce(
                    out=m_acc[:], in_=xg[(0, g)][:].transpose([0, 2, 1]),
                    axis=mybir.AxisListType.X, op=AL.add)
                nc.vector.tensor_reduce(
                    out=mt_acc[:], in_=xt[g][:].transpose([0, 2, 1]),
                    axis=mybir.AxisListType.X, op=AL.add)
            else:
                part = pool_tmp.tile([128, SLAB_W], f32, tag="mpart")
                nc.vector.tensor_reduce(
                    out=part[:], in_=xg[(0, g)][:].transpose([0, 2, 1]),
                    axis=mybir.AxisListType.X, op=AL.add)
                nc.vector.tensor_add(m_acc[:], m_acc[:], part[:])
                tpart = pool_tmp.tile([4, SLAB_W], f32, tag="mtpart")
                nc.vector.tensor_reduce(
                    out=tpart[:], in_=xt[g][:].transpose([0, 2, 1]),
                    axis=mybir.AxisListType.X, op=AL.add)
                nc.vector.tensor_add(mt_acc[:], mt_acc[:], tpart[:])

        # scaled mean m_s = (sum_c x) * s2c   (cst col 50)
        m_sA = pool_mean.tile([128, SLAB_W], f32)
        nc.vector.tensor_scalar_mul(out=m_sA[:], in0=m_acc[:], scalar1=cst[:, 50:51])
        m_sB = pool_mean.tile([4, SLAB_W], f32)
        nc.vector.tensor_scalar_mul(out=m_sB[:], in0=mt_acc[:], scalar1=cst[0:4, 50:51])

        # di-shifted views of m_s (rows di..di+128 of the slab)
        msd = {0: m_sA}
        for di in range(1, 5):
            t = pool_mean.tile([128, SLAB_W], f32, tag=f"msd{di}")
            nc.sync.dma_start(t[0:128 - di, :], m_sA[di:128, :])
            nc.sync.dma_start(t[128 - di:128, :], m_sB[0:di, :])
            msd[di] = t

        # ---- main tap loop ----
        acc = [pool_acc.tile([128, GC, W], f32, tag=f"acc{g}") for g in range(CG)]
        denom = pool_acc.tile([128, W], f32)
        first = True
        for di in range(5):
            if di > 0 and any((di * 5 + dj) in active_taps for dj in range(5)):
                for g in range(CG):
                    t = pool_x.tile([128, GC, SLAB_W], f32, tag=f"xg{g}")
                    nc.sync.dma_start(
                        t[:], xs_d[g * GC:(g + 1) * GC, di:di + 128, :].transpose([1, 0, 2]))
                    xg[(di, g)] = t
            for dj in range(5):
                t_idx = di * 5 + dj
                if t_idx not in active_taps:
                    continue
                d = pool_w.tile([128, W], f32, tag="d")
                nc.vector.tensor_tensor(
                    out=d[:], in0=msd[di][:, dj:dj + W], in1=msd[2][:, 2:2 + W],
                    op=AL.subtract)
                sq = pool_w.tile([128, W], f32, tag="sq")
                nc.scalar.activation(out=sq[:], in_=d[:], func=AF.Square)
                e = pool_w.tile([128, W], f32, tag="e")
                nc.scalar.activation(out=e[:], in_=sq[:], func=AF.Exp, scale=-1.0)
                Wt = pool_w.tile([128, W], f32, tag="Wt")
                nc.vector.tensor_scalar_mul(
                    out=Wt[:], in0=e[:], scalar1=cst[:, t_idx:t_idx + 1])
                if first:
                    nc.vector.tensor_scalar_mul(
                        out=denom[:], in0=e[:], scalar1=cst[:, 25 + t_idx:26 + t_idx])
                else:
                    nc.vector.scalar_tensor_tensor(
                        out=denom[:], in0=e[:], scalar=cst[:, 25 + t_idx:26 + t_idx],
                        in1=denom[:], op0=AL.mult, op1=AL.add)
                Wb = Wt[:].unsqueeze(1).broadcast_to([128, GC, W])
                for g in range(CG):
                    xsl = xg[(di, g)][:, :, dj:dj + W]
                    if first:
                        nc.vector.tensor_tensor(out=acc[g][:], in0=Wb, in1=xsl, op=AL.mult)
                    else:
                        prod = pool_tmp.tile([128, GC, W], f32, tag="prod")
                        nc.vector.tensor_tensor(out=prod[:], in0=Wb, in1=xsl, op=AL.mult)
                        nc.vector.tensor_add(acc[g][:], acc[g][:], prod[:])
                first = False

        # ---- normalize + store ----
        recip = pool_w.tile([128, W], f32, tag="recip")
        nc.vector.reciprocal(out=recip[:], in_=denom[:])
        rb = recip[:].unsqueeze(1).broadcast_to([128, GC, W])
        for g in range(CG):
            og = pool_tmp.tile([128, GC, W], f32, tag="prod")
            nc.vector.tensor_tensor(out=og[:], in0=acc[g][:], in1=rb, op=AL.mult)
            nc.sync.dma_start(
                out_d[g * GC:(g + 1) * GC, :, :].transpose([1, 0, 2]), og[:])

    nc.compile()
    return nc


def _prep_inputs(x, params):
    """Build per-core in_maps."""
    x = np.ascontiguousarray(x, dtype=np.float32)
    tap_consts = _host_tap_constants(params)
    active = set()
    for (sk_eff, sk2, s2c, is5) in tap_consts:
        active |= {t for t in range(NT) if is5 or sk_eff[t] != 0.0 or sk2[t] != 0.0}
    # pad whole batch once: [B, C, H+4, W+4]
    xp = np.pad(x, ((0, 0), (0, 0), (2, 2), (2, 2)))
    in_maps = []
    for core in range(NCORES):
        b, half = core // 2, core % 2
        h0 = half * HALF
        slab = np.ascontiguousarray(xp[b, :, h0:h0 + SLAB_H, :])
        sk_eff, sk2, s2c, _ = tap_consts[b]
        cst = np.zeros((128, 51), np.float32)
        cst[:, 0:25] = sk_eff[None, :]
        cst[:, 25:50] = sk2[None, :]
        cst[:, 50] = s2c
        in_maps.append({"xs": slab, "cst": cst})
    return in_maps, frozenset(active)


def kernel(x, params):
    from concourse.bass_utils import run_bass_kernel_spmd
    in_maps, active = _prep_inputs(x, params)
    key = ("nc", active)
    if key not in _CACHE:
        _CACHE[key] = _build(active)
    nc = _CACHE[key]
    res = run_bass_kernel_spmd(nc, in_maps, list(range(NCORES)))
    out = np.empty((B, C, H, W), np.float32)
    for core in range(NCORES):
        b, half = core // 2, core % 2
        out[b, :, half * HALF:(half + 1) * HALF, :] = res.results[core]["out"]
    return out


# revision 3
# speedup vs baseline: 1.0510x; 1.0510x over previous
"""Bilateral filter (nn_BilateralFilter) on 8 Trainium2 NeuronCores.

Sharding: data-parallel over (batch, H-half): core i -> sample i//2,
row-half i%2 (128 output rows each). Each core receives a host-padded
input slab [C, 132, 260] (2-row/2-col zero halos) plus per-sample tap
weights derived from `params` on the host; it computes the 5x5 (or
masked 3x3) bilateral filter for its 128x256 output tile.

Math (exact rewrite of the reference):
  out[c,p] = sum_t sk[t]*e_t[p]*x[c,p+t] / sum_t (sk[t]+1e-8*mask[t])*e_t[p]
  e_t[p]   = exp(-((m*s)[p+t] - (m*s)[p])^2),  s = 1/(sqrt(2)*sigma2)
where m is the channel-mean image and sk is the mask-folded normalized
spatial kernel. The 1e-8*mask term reproduces the reference's
`w/(w.sum()+1e-8)` epsilon after multiplying through by the color-kernel
normalizer.
"""

import numpy as np

B, C, H, W = 4, 32, 256, 256
HALF = H // 2          # output rows per core
SLAB_H = HALF + 4      # input rows incl. 2-row halos
SLAB_W = W + 4         # input cols incl. 2-col halos
NCORES = 8
NT = 25                # 5x5 taps
CG = 4                 # channel groups
GC = C // CG           # channels per group

_CACHE = {}


def _host_tap_constants(params):
    """Per-sample sk_eff[25], sk2[25], s2c scalar (all float32 math)."""
    p = params.astype(np.float32)
    sig = (1.0 / (1.0 + np.exp(-p))).astype(np.float32)
    coords = (np.arange(5, dtype=np.float32) - 2.0)
    grid = coords[:, None] ** 2 + coords[None, :] ** 2
    center3 = ((np.abs(coords)[:, None] <= 1) & (np.abs(coords)[None, :] <= 1)).astype(np.float32)
    out = []
    for b in range(B):
        k_raw = np.float32(1.0) + np.float32(2.0) * sig[b, 0]
        is5 = bool(k_raw >= 2.0)
        sigma1 = np.float32(3.5) + np.float32(5.5) * sig[b, 1]
        sigma2 = np.float32(5.5) + np.float32(7.5) * sig[b, 2]
        mask = np.ones((5, 5), np.float32) if is5 else center3
        sk = np.exp(-grid / (2.0 * sigma1 ** 2)).astype(np.float32) * mask
        sk = (sk / sk.sum()).astype(np.float32)
        sk_eff = sk.reshape(NT)
        sk2 = (sk_eff + np.float32(1e-8) * mask.reshape(NT)).astype(np.float32)
        # m_s = (sum_c x) * s2c  ==  mean * (1/(sqrt(2)*sigma2))
        s2c = np.float32(1.0 / (np.sqrt(2.0, dtype=np.float64) * float(sigma2)) / C)
        out.append((sk_eff, sk2, s2c, is5))
    return out


def _build(active_taps, n_iter=1):
    from contextlib import ExitStack
    import concourse.tile as tile
    from concourse import bacc, mybir

    f32 = mybir.dt.float32
    AF = mybir.ActivationFunctionType
    AL = mybir.AluOpType

    nc = bacc.Bacc("TRN2", target_bir_lowering=False, debug=False,
                   num_devices=NCORES)
    xs_d = nc.dram_tensor("xs", [C, SLAB_H, SLAB_W], f32, kind="ExternalInput").ap()
    cst_d = nc.dram_tensor("cst", [128, 51], f32, kind="ExternalInput").ap()
    out_d = nc.dram_tensor("out", [C, HALF, W], f32, kind="ExternalOutput").ap()

    from contextlib import nullcontext
    with tile.TileContext(nc) as tc, ExitStack() as ctx:
        loop_ctx = tc.For_i(0, n_iter, 1) if n_iter > 1 else nullcontext()
        pool_c = ctx.enter_context(tc.tile_pool(name="cstp", bufs=1))
        pool_x = ctx.enter_context(tc.tile_pool(name="xp", bufs=2))
        pool_mean = ctx.enter_context(tc.tile_pool(name="meanp", bufs=1))
        pool_w = ctx.enter_context(tc.tile_pool(name="wp", bufs=3))
        pool_acc = ctx.enter_context(tc.tile_pool(name="accp", bufs=1))
        pool_tmp = ctx.enter_context(tc.tile_pool(name="tmpp", bufs=3))

        cst = pool_c.tile([128, 51], f32, name="cst")
        nc.sync.dma_start(cst[:], cst_d)
        ctx.enter_context(loop_ctx)

        # ---- load x slabs for di=0 (also feeds the mean) + tail rows ----
        xg = {}
        for g in range(CG):
            t = pool_x.tile([128, GC, SLAB_W], f32, tag=f"xg{g}", name=f"xg0_{g}")
            nc.sync.dma_start(
                t[:], xs_d[g * GC:(g + 1) * GC, 0:128, :].transpose([1, 0, 2]))
            xg[(0, g)] = t
        xt = []
        for g in range(CG):
            t = pool_mean.tile([4, GC, SLAB_W], f32, tag=f"xt{g}", name=f"xt{g}")
            nc.sync.dma_start(
                t[:], xs_d[g * GC:(g + 1) * GC, 128:132, :].transpose([1, 0, 2]))
            xt.append(t)

        # ---- channel mean (rows 0..128 and tail rows 128..132) ----
        m_acc = pool_mean.tile([128, SLAB_W], f32, name="m_acc")
        mt_acc = pool_mean.tile([4, SLAB_W], f32, name="mt_acc")
        for g in range(CG):
            if g == 0:
                nc.vector.tensor_reduce(
                    out=m_acc[:], in_=xg[(0, g)][:].transpose([0, 2, 1]),
                    axis=mybir.AxisListType.X, op=AL.add)
                nc.vector.tensor_reduce(
                    out=mt_acc[:], in_=xt[g][:].transpose([0, 2, 1]),
                    axis=mybir.AxisListType.X, op=AL.add)
            else:
                part = pool_tmp.tile([128, SLAB_W], f32, tag="mpart", name=f"mpart{g}")
                nc.vector.tensor_reduce(
                    out=part[:], in_=xg[(0, g)][:].transpose([0, 2, 1]),
                    axis=mybir.AxisListType.X, op=AL.add)
                nc.vector.tensor_add(m_acc[:], m_acc[:], part[:])
                tpart = pool_tmp.tile([4, SLAB_W], f32, tag="mtpart", name=f"mtpart{g}")
                nc.vector.tensor_reduce(
                    out=tpart[:], in_=xt[g][:].transpose([0, 2, 1]),
                    axis=mybir.AxisListType.X, op=AL.add)
                nc.vector.tensor_add(mt_acc[:], mt_acc[:], tpart[:])

        # scaled mean m_s = (sum_c x) * s2c   (cst col 50)
        m_sA = pool_mean.tile([128, SLAB_W], f32, name="m_sA")
        nc.vector.tensor_scalar_mul(out=m_sA[:], in0=m_acc[:], scalar1=cst[:, 50:51])
        m_sB = pool_mean.tile([4, SLAB_W], f32, name="m_sB")
        nc.vector.tensor_scalar_mul(out=m_sB[:], in0=mt_acc[:], scalar1=cst[0:4, 50:51])

        # di-shifted views of m_s (rows di..di+128 of the slab)
        msd = {0: m_sA}
        for di in range(1, 5):
            t = pool_mean.tile([128, SLAB_W], f32, tag=f"msd{di}", name=f"msd{di}")
            nc.sync.dma_start(t[0:128 - di, :], m_sA[di:128, :])
            nc.sync.dma_start(t[128 - di:128, :], m_sB[0:di, :])
            msd[di] = t

        # ---- main tap loop ----
        acc = [pool_acc.tile([128, GC, W], f32, tag=f"acc{g}", name=f"acc{g}") for g in range(CG)]
        denom = pool_acc.tile([128, W], f32, name="denom")
        first = True
        for di in range(5):
            if di > 0 and any((di * 5 + dj) in active_taps for dj in range(5)):
                for g in range(CG):
                    t = pool_x.tile([128, GC, SLAB_W], f32, tag=f"xg{g}", name=f"xg{di}_{g}")
                    nc.sync.dma_start(
                        t[:], xs_d[g * GC:(g + 1) * GC, di:di + 128, :].transpose([1, 0, 2]))
                    xg[(di, g)] = t
            for dj in range(5):
                t_idx = di * 5 + dj
                if t_idx not in active_taps:
                    continue
                d = pool_w.tile([128, W], f32, tag="d", name=f"d{t_idx}")
                nc.vector.tensor_tensor(
                    out=d[:], in0=msd[di][:, dj:dj + W], in1=msd[2][:, 2:2 + W],
                    op=AL.subtract)
                sq = pool_w.tile([128, W], f32, tag="sq", name=f"sq{t_idx}")
                nc.scalar.activation(out=sq[:], in_=d[:], func=AF.Square)
                e = pool_w.tile([128, W], f32, tag="e", name=f"e{t_idx}")
                nc.scalar.activation(out=e[:], in_=sq[:], func=AF.Exp, scale=-1.0)
                Wt = pool_w.tile([128, W], f32, tag="Wt", name=f"Wt{t_idx}")
                nc.vector.tensor_scalar_mul(
                    out=Wt[:], in0=e[:], scalar1=cst[:, t_idx:t_idx + 1])
                if first:
                    nc.vector.tensor_scalar_mul(
                        out=denom[:], in0=e[:], scalar1=cst[:, 25 + t_idx:26 + t_idx])
                else:
                    nc.vector.scalar_tensor_tensor(
                        out=denom[:], in0=e[:], scalar=cst[:, 25 + t_idx:26 + t_idx],
                        in1=denom[:], op0=AL.mult, op1=AL.add)
                Wb = Wt[:].unsqueeze(1).broadcast_to([128, GC, W])
                for g in range(CG):
                    xsl = xg[(di, g)][:, :, dj:dj + W]
                    if first:
                        nc.vector.tensor_tensor(out=acc[g][:], in0=Wb, in1=xsl, op=AL.mult)
                    else:
                        prod = pool_tmp.tile([128, GC, W], f32, tag="prod", name=f"prod{t_idx}_{g}")
                        nc.vector.tensor_tensor(out=prod[:], in0=Wb, in1=xsl, op=AL.mult)
                        nc.vector.tensor_add(acc[g][:], acc[g][:], prod[:])
                first = False

        # ---- normalize + store ----
        recip = pool_w.tile([128, W], f32, tag="recip", name="recip")
        nc.vector.reciprocal(out=recip[:], in_=denom[:])
        rb = recip[:].unsqueeze(1).broadcast_to([128, GC, W])
        for g in range(CG):
            og = pool_tmp.tile([128, GC, W], f32, tag="prod", name=f"og{g}")
            nc.vector.tensor_tensor(out=og[:], in0=acc[g][:], in1=rb, op=AL.mult)
            nc.sync.dma_start(
                out_d[g * GC:(g + 1) * GC, :, :].transpose([1, 0, 2]), og[:])

    nc.compile()
    return nc


def _prep_inputs(x, params):
    """Build per-core in_maps."""
    x = np.ascontiguousarray(x, dtype=np.float32)
    tap_consts = _host_tap_constants(params)
    active = set()
    for (sk_eff, sk2, s2c, is5) in tap_consts:
        active |= {t for t in range(NT) if is5 or sk_eff[t] != 0.0 or sk2[t] != 0.0}
    # pad whole batch once: [B, C, H+4, W+4]
    xp = np.pad(x, ((0, 0), (0, 0), (2, 2), (2, 2)))
    in_maps = []
    for core in range(NCORES):
        b, half = core // 2, core % 2
        h0 = half * HALF
        slab = np.ascontiguousarray(xp[b, :, h0:h0 + SLAB_H, :])
        sk_eff, sk2, s2c, _ = tap_consts[b]
        cst = np.zeros((128, 51), np.float32)
        cst[:, 0:25] = sk_eff[None, :]
        cst[:, 25:50] = sk2[None, :]
        cst[:, 50] = s2c
        in_maps.append({"xs": slab, "cst": cst})
    return in_maps, frozenset(active)


def kernel(x, params, n_iter=1):
    from concourse.bass_utils import run_bass_kernel_spmd
    in_maps, active = _prep_inputs(x, params)
    key = ("nc", active, n_iter)
    if key not in _CACHE:
        _CACHE[key] = _build(active, n_iter)
    nc = _CACHE[key]
    res = run_bass_kernel_spmd(nc, in_maps, list(range(NCORES)))
    out = np.empty((B, C, H, W), np.float32)
    for core in range(NCORES):
        b, half = core // 2, core % 2
        out[b, :, half * HALF:(half + 1) * HALF, :] = res.results[core]["out"]
    return out
